# revision 1
# baseline (speedup 1.0000x reference)
"""Adaptive-softmax CE loss on 8 TRN2 NeuronCores.

Strategy: data-parallel over tokens (4096 tokens -> 512/core), weights
replicated, no collectives.  Per core the Bass/Tile kernel computes
  - hT0 = tail0_w1 @ w_in_shard.T, hT1 = tail1_w1 @ w_in_shard.T
    (fp8 DoubleRow matmuls, x64-scaled weights, unscaled in the PSUM copy)
  - label logits via elementwise-mul + ones-vector matmul (partition reduce)
  - streamed vocab-chunk logit matmuls (fp8 DoubleRow, 2x PE rate; the x64
    weight scale is undone for free via exp(x/64) on ScalarE) with fused
    exp + per-token row-sum
and exports per-token sum-exp partials + label dots.  The host finishes
with log() in float64, applies the cluster masks, and averages.

The exp engine (ScalarE) is the bottleneck: 25.6M logits/core at
1 elem/lane/cycle = 167us floor, ~209us with per-instruction overhead.
The kernel therefore runs the ACT-heavy tail1 stream (K=256: cheap on
PE) as continuous filler starting ~15us in - it only needs the two
tail1 h-chunks - and sprinkles every PE-heavy item (rest of phase A,
head tiles, label-dot phase B, tail0 tiles) into the PE slack between
tail1 tiles.  PE-heavy head/tail0 work is emitted as two ~1024-wide
PSUM tiles each so no contiguous PE stretch outruns the 2-deep PSUM
rotation that feeds ScalarE.  Per-token sums are split between ScalarE
(activation accum_out, cheap) and VectorE (exp->bf16 + row-reduce) to
keep VectorE ~50us under ScalarE (tighter balance couples the engines
through the shallow buffers and costs more than it saves).
Measured: ~255us HW exec on 8 cores (vs 380us bf16 baseline).

Numerics: fp8 logit noise ~0.05 abs gives a logsumexp convexity bias of
~1e-4 absolute on a loss of ~18 (~1e-5 rel); label dots are bf16 products
of fp8 h / bf16 gathers.  max|logit| < 6 so no max-subtraction is needed.
"""

import numpy as np
import ml_dtypes

CUTOFF = [2000, 10000, 50000]
N_TOK = 4096
D = 1024
N_CORES = 8
TOK_PER_CORE = N_TOK // N_CORES          # 512
N_BLK = TOK_PER_CORE // 128              # 4 token blocks of 128
KX = 9                                   # k-chunks of augmented input (1152/128)
K0 = 8                                   # k-chunks of tail0 proj (1024/128)
K1 = 2                                   # k-chunks of tail1 proj (256/128)
SUP = 2048                               # ACT super-chunk width
N_HEAD = CUTOFF[0] + 2                   # 2002
N_T0 = CUTOFF[1] - CUTOFF[0]             # 8000
N_T1 = CUTOFF[2] - CUTOFF[1]             # 40000
N_SUP0 = (N_T0 + SUP - 1) // SUP         # 4 (last 1856)
N_SUP1 = (N_T1 + SUP - 1) // SUP         # 20 (last 1088)
S_COLS = N_BLK * 2 + 16 * 2 + 80         # 120: head/t0 2-slice cols + t1
WSCALE = 64.0                            # fp8 weight pre-scale (undone in exp)

BF16 = ml_dtypes.bfloat16
FP8 = ml_dtypes.float8_e4m3

_cache = {}


def _sup_w(sup, total):
    return min(SUP, total - sup * SUP)


def _subs(width):
    out, o = [], 0
    while o < width:
        out.append((o, min(512, width - o)))
        o += min(512, width - o)
    return out


def _schedule():
    """Emission order + out_s column map. Pure data, shared with the host."""
    sched = [("fetch_t1", 0), ("a1", 0), ("a1", 1)]
    cols, acc = [], []
    n_t1 = 0

    def col(kind, b, use_accum):
        cols.append((kind, b))
        acc.append(use_accum)

    for s in range(N_SUP1):
        if s >= 1:
            sched.append(("fetch_t1", s))
        heavies = []
        if s == 0:
            heavies = [("fetchw", "w1t0"), ("a2", 0), ("a2", 1)]
        elif s == 1:
            heavies = [("fetchw", "hwt8"), ("fetchw", "hbias"), ("a2", 2), ("a2", 3)]
        elif s == 2:
            heavies = [("a2", 4), ("a2", 5), ("head", 0, 0), ("head", 0, 1)]
        elif s == 3:
            heavies = [("a2", 6), ("a2", 7), ("fetch_t0", 0)]
        elif s == 6:
            heavies = [("head", 1, 0), ("head", 1, 1)]
        elif s == 9:
            heavies = [("head", 2, 0), ("head", 2, 1)]
        elif s == 10:
            heavies = [("Bmm", 0)]
        elif s == 12:
            heavies = [("Bmm", 1)]
        elif s == 14:
            heavies = [("head", 3, 0), ("head", 3, 1)]
        if 2 <= s <= 7:
            # 28 small DMA chunks of xt/gall sprinkled so no bulk transfer
            # blocks the serial SP HWDGE ring; chunk id 9+i (gall chunk i)
            # must land before ("Bmul", i) at sup 5 + i//5
            lo, hi = [(0, 6), (6, 12), (12, 18), (18, 22), (22, 25), (25, 28)][s - 2]
            heavies += [("fetchBchunk", i) for i in range(lo, hi)]
        if 5 <= s <= 8:
            heavies += [("Bmul", i) for i in range(5 * (s - 5), min(5 * (s - 4), 19))]
        if 4 <= s <= 19:
            r = (s - 4) // 4
            b0 = (s - 4) % 4
            heavies.append(("t0", r, b0, 0))
            heavies.append(("t0", r, b0, 1))
            if b0 == 3 and r < 3:
                heavies.append(("fetch_t0", r + 1))
        for j in range(4):
            if j < len(heavies):
                sched.append(heavies[j])
            sched.append(("t1", s, j))
        for h in heavies[4:]:
            sched.append(h)

    for item in sched:
        if item[0] == "head":
            col("h", item[1], False)       # one col per half-tile
        elif item[0] == "t0":
            col("t0", item[2], False)
        elif item[0] == "t1":
            col("t1", item[2], n_t1 < 16 or n_t1 % 2 == 0 or n_t1 >= 74)
            n_t1 += 1
    assert len(cols) == S_COLS
    return sched, cols, acc


def _build_nc():
    import concourse.bass as bass
    import concourse.bacc as bacc
    import concourse.mybir as mybir
    from concourse import tile

    dt = mybir.dt
    nc = bacc.Bacc(None)

    sched, cols, acc = _schedule()
    _cache["cols"] = cols

    xt8_p = nc.declare_dram_parameter("xt8", [K0, 128, TOK_PER_CORE], dt.float8e4, isOutput=False)
    w1t0_p = nc.declare_dram_parameter("w1t0", [K0, 128, 1024], dt.float8e4, isOutput=False)
    w1t1_p = nc.declare_dram_parameter("w1t1", [K0, 128, 256], dt.float8e4, isOutput=False)
    hwt8_p = nc.declare_dram_parameter("hwt8", [K0, 128, N_HEAD], dt.float8e4, isOutput=False)
    hbias_p = nc.declare_dram_parameter("hbias", [1, N_HEAD], dt.bfloat16, isOutput=False)
    xt_p = nc.declare_dram_parameter("xt", [KX, 128, TOK_PER_CORE], dt.bfloat16, isOutput=False)
    gall_p = nc.declare_dram_parameter("gall", [KX + K0 + K1, 128, TOK_PER_CORE], dt.bfloat16, isOutput=False)
    w2t0_p = nc.declare_dram_parameter("w2t0", [K0, 128, N_T0], dt.float8e4, isOutput=False)
    w2t1_p = nc.declare_dram_parameter("w2t1", [K1, 128, N_T1], dt.float8e4, isOutput=False)
    out_s_p = nc.declare_dram_parameter("out_s", [128, S_COLS], dt.float32, isOutput=True)
    out_ll_p = nc.declare_dram_parameter("out_ll", [1, TOK_PER_CORE], dt.float32, isOutput=True)

    EXP = mybir.ActivationFunctionType.Exp
    MULT = mybir.AluOpType.mult
    ADD = mybir.AluOpType.add
    DR = mybir.MatmulPerfMode.DoubleRow
    X = mybir.AxisListType.X
    PSUM = bass.MemorySpace.PSUM

    def dma3(dst_tile, src_3d):
        # one DMA per tensor: DRAM [C,128,F] -> SBUF [128,C,F]
        nc.sync.dma_start(dst_tile[:], src_3d.rearrange("c p t -> p c t"))

    with tile.TileContext(nc) as tc:
        with (
            tc.tile_pool(name="res", bufs=1) as res,
            tc.tile_pool(name="w2s0", bufs=3) as w2s0,
            tc.tile_pool(name="w2s1", bufs=4) as w2s1,
            tc.tile_pool(name="prs", bufs=19) as prs,
            tc.tile_pool(name="es", bufs=5) as es,
        ):
            # ---- resident SBUF tensors, DMA'd in dependency-critical order ----
            xt8 = res.tile([128, K0, TOK_PER_CORE], dt.float8e4, tag="xt8")
            w1t0 = res.tile([128, K0, 1024], dt.float8e4, tag="w1t0")
            w1t1 = res.tile([128, K0, 256], dt.float8e4, tag="w1t1")
            hwt8 = res.tile([128, K0, N_HEAD], dt.float8e4, tag="hwt8")
            hbias = res.tile([1, N_HEAD], dt.bfloat16, tag="hbias")
            xt = res.tile([128, KX, TOK_PER_CORE], dt.bfloat16, tag="xt")
            gt = res.tile([128, KX + K0 + K1, TOK_PER_CORE], dt.bfloat16, tag="g")
            ht0_8 = res.tile([128, K0, TOK_PER_CORE], dt.float8e4, tag="ht0_8")
            ht1_8 = res.tile([128, K1, TOK_PER_CORE], dt.float8e4, tag="ht1_8")
            sall = res.tile([128, S_COLS], dt.float32, tag="sall")
            ll = res.tile([1, TOK_PER_CORE], dt.float32, tag="ll")
            ones = res.tile([128, 1], dt.bfloat16, tag="ones")
            ones1 = res.tile([1, 128], dt.bfloat16, tag="ones1")

            nc.gpsimd.memset(ones[:], 1.0)
            nc.gpsimd.memset(ones1[:], 1.0)
            dma3(xt8, xt8_p)
            dma3(w1t1, w1t1_p)

            _t0w, _t1w = {}, {}
            col_iter = iter(range(S_COLS))

            with tc.tile_pool(name="pc", bufs=2, space=PSUM) as pcp:

                def exp_reduce(pc, off, w, col):
                    if acc[col]:
                        nc.scalar.activation(
                            pc[:, off:off + w], pc[:, off:off + w], EXP,
                            scale=1.0 / WSCALE, accum_out=sall[:, col:col + 1],
                        )
                    else:
                        et = es.tile([128, SUP], dt.bfloat16, tag="e")
                        nc.scalar.activation(et[:, :w], pc[:, off:off + w], EXP,
                                             scale=1.0 / WSCALE)
                        nc.vector.tensor_reduce(sall[:, col:col + 1], et[:, :w],
                                                axis=X, op=ADD)

                def mm_group(pc, sl, b, kk, lhs3, rhs3, bias=False, rbase=0):
                    rsl = slice(rbase + sl.start, rbase + sl.stop)
                    for c in range(kk // 2):
                        nc.tensor.matmul(
                            pc[:, sl],
                            lhsT=lhs3[:, 2 * c:2 * c + 2, b * 128:(b + 1) * 128],
                            rhs=rhs3[:, 2 * c:2 * c + 2, rsl],
                            start=(c == 0),
                            stop=(c == kk // 2 - 1 and not bias),
                            perf_mode=DR,
                        )
                    if bias:
                        # K=1: ones column x (64*head_b) row, bf16
                        nc.tensor.matmul(pc[:, sl], lhsT=ones1[:],
                                         rhs=hbias[0:1, rsl], start=False, stop=True)

                def a_tile(w1t, m, ht_8):
                    pt = pcp.tile([128, SUP], dt.float32, tag="pc")
                    for c in range(K0 // 2):
                        nc.tensor.matmul(
                            pt[:, :TOK_PER_CORE],
                            lhsT=w1t[:, 2 * c:2 * c + 2, m * 128:(m + 1) * 128],
                            rhs=xt8[:, 2 * c:2 * c + 2, :],
                            start=(c == 0), stop=(c == K0 // 2 - 1),
                            perf_mode=DR,
                        )
                    nc.vector.tensor_scalar_mul(ht_8[:, m, :],
                                                pt[:, :TOK_PER_CORE], 1.0 / WSCALE)

                lhs_chunks = (
                    [xt[:, k, :] for k in range(KX)]
                    + [ht0_8[:, k, :] for k in range(K0)]
                    + [ht1_8[:, k, :] for k in range(K1)]
                )
                _prs = {}

                def emit_Bmul(i):
                    pr = prs.tile([128, TOK_PER_CORE], dt.bfloat16, tag="pr")
                    nc.vector.tensor_tensor(pr[:], lhs_chunks[i], gt[:, i, :], op=MULT)
                    _prs[i] = pr

                def emit_Bmm(half):
                    n = len(lhs_chunks)
                    rng = range(0, n // 2) if half == 0 else range(n // 2, n)
                    pll_t = pcp.tile([128, SUP], dt.float32, tag="pc")
                    pll = pll_t[0:1, :TOK_PER_CORE]
                    for i in rng:
                        nc.tensor.matmul(pll, lhsT=ones[:], rhs=_prs[i][:],
                                         start=(i == rng.start), stop=(i == rng.stop - 1))
                    if half == 0:
                        nc.vector.tensor_copy(ll[:], pll)
                    else:
                        # ll += chain1 (SBUF in0, PSUM in1)
                        nc.vector.tensor_tensor(ll[:], ll[:], pll, op=ADD)

                for item in sched:
                    kind = item[0]
                    if kind == "fetch_t1":
                        sup = item[1]
                        wt = w2s1.tile([128, K1, SUP], dt.float8e4, tag="w1")
                        w = _sup_w(sup, N_T1)
                        nc.sync.dma_start(
                            wt[:, :, :w],
                            w2t1_p[:, :, sup * SUP:sup * SUP + w].rearrange("c p t -> p c t"),
                        )
                        _t1w[sup] = wt
                    elif kind == "fetch_t0":
                        sup = item[1]
                        wt = w2s0.tile([128, K0, SUP], dt.float8e4, tag="w0")
                        w = _sup_w(sup, N_T0)
                        nc.sync.dma_start(
                            wt[:, :, :w],
                            w2t0_p[:, :, sup * SUP:sup * SUP + w].rearrange("c p t -> p c t"),
                        )
                        _t0w[sup] = wt
                    elif kind == "fetchw":
                        if item[1] == "w1t0":
                            dma3(w1t0, w1t0_p)
                        elif item[1] == "hwt8":
                            dma3(hwt8, hwt8_p)
                        else:
                            nc.sync.dma_start(hbias[:], hbias_p[:])
                    elif kind == "fetchBchunk":
                        i = item[1]
                        if i < KX:
                            nc.sync.dma_start(xt[:, i, :], xt_p[i])
                        else:
                            nc.sync.dma_start(gt[:, i - KX, :], gall_p[i - KX])
                    elif kind == "a1":
                        a_tile(w1t1, item[1], ht1_8)
                    elif kind == "a2":
                        a_tile(w1t0, item[1], ht0_8)
                    elif kind == "Bmul":
                        emit_Bmul(item[1])
                    elif kind == "Bmm":
                        emit_Bmm(item[1])
                    elif kind == "head":
                        # PE-heavy tiles split into two narrow PSUM tiles so
                        # no contiguous PE stretch exceeds ~2us
                        _, b, hf = item
                        base = hf * 1024
                        width = min(1024, N_HEAD - base)
                        pc = pcp.tile([128, SUP], dt.float32, tag="pc")
                        for off, w in _subs(width):
                            mm_group(pc, slice(off, off + w), b, K0, xt8,
                                     hwt8, bias=True, rbase=base)
                        exp_reduce(pc, 0, width, next(col_iter))
                    elif kind == "t0":
                        _, r, b, hf = item
                        base = hf * 1024
                        width = min(1024, _sup_w(r, N_T0) - base)
                        pc = pcp.tile([128, SUP], dt.float32, tag="pc")
                        for off, w in _subs(width):
                            mm_group(pc, slice(off, off + w), b, K0, ht0_8,
                                     _t0w[r], rbase=r * SUP % SUP + base)
                        exp_reduce(pc, 0, width, next(col_iter))
                    elif kind == "t1":
                        _, sup, b = item
                        pc = pcp.tile([128, SUP], dt.float32, tag="pc")
                        width = _sup_w(sup, N_T1)
                        for off, w in _subs(width):
                            mm_group(pc, slice(off, off + w), b, K1, ht1_8, _t1w[sup])
                        exp_reduce(pc, 0, width, next(col_iter))

            nc.sync.dma_start(out_s_p[:], sall[:])
            nc.sync.dma_start(out_ll_p[:], ll[:])

    nc.compile()
    return nc


def _prep_inputs(w_in, target, head_w, head_b, tail0_w1, tail0_w2, tail1_w1, tail1_w2):
    """Host-side shard + transpose + cast. Returns in_maps + masks."""
    f32 = np.float32
    w_in = np.asarray(w_in, f32)
    target = np.asarray(target).astype(np.int64)
    head_w = np.asarray(head_w, f32)
    head_b = np.asarray(head_b, f32)
    t0w1 = np.asarray(tail0_w1, f32)
    t0w2 = np.asarray(tail0_w2, f32)
    t1w1 = np.asarray(tail1_w1, f32)
    t1w2 = np.asarray(tail1_w2, f32)

    c0, c1, c2 = CUTOFF
    mask0 = (target >= c0) & (target < c1)
    mask1 = (target >= c1) & (target < c2)
    label0 = np.clip(target - c0, 0, c1 - c0 - 1)
    label1 = np.clip(target - c1, 0, c2 - c1 - 1)
    first_t = np.where(mask0, c0, np.where(mask1, c0 + 1, target))

    # label-gathered rows, masks folded in
    g0 = t0w2[label0] * mask0[:, None].astype(f32)     # [N_TOK, 1024]
    g1 = t1w2[label1] * mask1[:, None].astype(f32)     # [N_TOK, 256]
    gh = head_w[first_t]                               # [N_TOK, 1024]
    bh = head_b[first_t]                               # [N_TOK]

    def chunks(a, k, dtype=BF16):  # [K*128, F] -> [K, 128, F]
        return np.ascontiguousarray(a.reshape(k, 128, a.shape[1])).astype(dtype)

    w1t0 = chunks(t0w1.T * WSCALE, K0, FP8)            # [8,128,1024] fp8
    w1t1 = chunks(t1w1.T * WSCALE, K0, FP8)            # [8,128,256] fp8
    w2t0 = chunks(t0w2.T * WSCALE, K0, FP8)            # [8,128,8000] fp8
    w2t1 = chunks(t1w2.T * WSCALE, K1, FP8)            # [2,128,40000] fp8
    hwt8 = chunks(head_w.T * WSCALE, K0, FP8)          # [8,128,2002] fp8
    hbias = (head_b[None, :] * WSCALE).astype(BF16)   # pairs with ones1 column

    in_maps = []
    for c in range(N_CORES):
        sl = slice(c * TOK_PER_CORE, (c + 1) * TOK_PER_CORE)
        xt = np.zeros((KX * 128, TOK_PER_CORE), f32)
        xt[:D] = w_in[sl].T
        xt[D] = 1.0                                    # augmented ones-row (bias)
        ght = np.zeros((KX * 128, TOK_PER_CORE), f32)
        ght[:D] = gh[sl].T
        ght[D] = bh[sl]
        gall = np.concatenate(
            [chunks(ght, KX), chunks(g0[sl].T, K0), chunks(g1[sl].T, K1)], axis=0
        )
        in_maps.append({
            "xt": chunks(xt, KX),
            "xt8": chunks(xt[:D], K0, FP8),
            "w1t0": w1t0, "w1t1": w1t1, "w2t0": w2t0, "w2t1": w2t1,
            "hwt8": hwt8, "hbias": hbias,
            "gall": gall,
        })
    return in_maps, mask0, mask1


def _combine(results, mask0, mask1):
    """Host-side unshard using the emission-order column map."""
    cols = _cache["cols"]
    total = 0.0
    for c in range(N_CORES):
        S = results[c]["out_s"].astype(np.float64)     # [128, S_COLS]
        llv = results[c]["out_ll"].astype(np.float64).reshape(N_BLK, 128)
        Sh = np.zeros((128, N_BLK))
        S0 = np.zeros((128, N_BLK))
        S1 = np.zeros((128, N_BLK))
        for j, (k, b) in enumerate(cols):
            if k == "h":
                Sh[:, b] += S[:, j]
            elif k == "t0":
                S0[:, b] += S[:, j]
            else:
                S1[:, b] += S[:, j]
        # token (p, b) -> global index c*512 + b*128 + p
        idx = (c * TOK_PER_CORE + np.arange(N_BLK)[None, :] * 128
               + np.arange(128)[:, None])
        m0 = mask0[idx]
        m1 = mask1[idx]
        nll = np.log(Sh) + m0 * np.log(S0) + m1 * np.log(S1) - llv.T
        total += nll.sum()
    return np.float32(total / N_TOK)


def _run(inputs, trace=False):
    from concourse.bass_utils import run_bass_kernel_spmd

    if "nc" not in _cache:
        _cache["nc"] = _build_nc()
    nc = _cache["nc"]
    in_maps, mask0, mask1 = _prep_inputs(**inputs)
    res = run_bass_kernel_spmd(nc, in_maps, core_ids=list(range(N_CORES)), trace=trace)
    loss = _combine(res.results, mask0, mask1)
    return loss, res


def kernel(**inputs) -> np.ndarray:
    loss, _ = _run(inputs, trace=False)
    return loss

